# revision 9
# baseline (speedup 1.0000x reference)
"""CrossTransformer (KNN message passing) Trainium2 kernel.

Contract: kernel(**inputs) takes the FULL unsharded inputs (numpy arrays,
keys as in setup_inputs()) and returns the FULL [2, 256, 2048] float32
output.  Internally shards across 8 NeuronCores: core = b*4 + s handles
batch b, key-point shard s (512 points), with the fused KNN database
replicated per core.

Pipeline per core (v2 — dma_gather edition):
  1. KNN scores S = 2*k.f - |f|^2 via a 21-row bf16 mantissa-split matmul
     (fp32-grade scores so the selected neighbor sets match the fp32
     reference exactly); top-16 via DVE max/max_index/match_replace
     (two top-8 rounds), indices straight to uint16.
  2. Index tile [128 pts, 16 slots] -> broadcast x8 along free ->
     HWDGE dma_start_transpose -> [128, 128] (transposed + replicated
     across partition groups), then ONE SWDGE dma_gather(transpose=True)
     per tile pulls 2048 rows of the [4096, 384] bf16 database
     (256 feat | 3 pcd | 125 zero-pad) LANDING CHANNEL-MAJOR:
     G3[p, ch, j*16+k] = db[idx[j,k], 128*ch+p].  This replaces the 16
     indirect row-gathers + 48 PE transposes + 12 DVE copies per tile of
     v1.  Pair layout is k-INNER (col = j*16 + k).
  3. pos/attn MLPs in bf16 with fp32 PSUM accumulation (BatchNorm folded
     into the weights host-side; pos_b2 applied as the pe-copy activation
     bias so pe == reference pos_embedding), exp without max-subtraction
     (logits are tiny).
  4. Per-channel softmax over the 16 contiguous (k-inner) neighbors via
     bf16 fold chains + fast-reciprocal, fused with the weighted sum,
     done per 1024-col slice-pair so the tail overlaps the matmul stream.
"""

import copy as _copy

import numpy as np

import concourse.bass as bass
import concourse.mybir as mybir
import concourse.tile as tile
from concourse import bass_utils
from concourse import library_config

F32 = mybir.dt.float32
BF16 = mybir.dt.bfloat16
U16 = mybir.dt.uint16
I16 = mybir.dt.int16
AF = mybir.ActivationFunctionType

B = 2
C = 256
N = 2048
M = 2048
F = N + M            # fused database size
KNN = 16
PH = 64              # pos MLP hidden
AH = 1024            # attn MLP hidden
P = 128
NCORES = 8
SHARD = N * B // NCORES      # 512 key points per core
NT = SHARD // P              # 4 point-tiles per core
ROW = 384                    # db row: 256 feat + 3 pcd + 125 pad (768B)
SL = 512                     # matmul free-dim slice
NSL = P * KNN // SL          # 4 slices per point-tile
BN_EPS = 1e-5
NEG_BIG = -3.0e38

# Module-level knobs for test harnesses (not used by the grader).
TRACE = False
LAST_RESULT = None


def _legalize_sync_waits(nc, max_waits=1):
    """walrus here accepts at most one sync wait per instruction; move
    extra waits onto ENGINE_NOP carriers inserted just before the offender
    (same engine: the sequencer accumulates the waits, no pipeline drain)."""
    module = nc.m
    new_module = _copy.replace(module, functions=[])
    for function in module.functions:
        new_function = _copy.replace(function, blocks=[])
        new_function.set_allocations_from_list(function.allocations)
        for block in function.blocks:
            out = []
            for inst in block.instructions:
                si = inst.sync_info
                waits = list(si.on_wait) if si is not None else []
                if len(waits) > max_waits:
                    extra, keep = waits[:-max_waits], waits[-max_waits:]
                    for j in range(0, len(extra), max_waits):
                        out.append(mybir.InstDrain(
                            name=f"I-lgl-{inst.name}-{j}",
                            engine=inst.engine,
                            ins=[], outs=[],
                            sync_info=mybir.SyncInfo(
                                on_wait=extra[j:j + max_waits], on_update=[]),
                        ))
                    inst.sync_info = mybir.SyncInfo(
                        on_wait=keep, on_update=list(si.on_update))
                out.append(inst)
            new_function.blocks.append(_copy.replace(block, instructions=out))
        new_module.functions.append(new_function)
    nc.m = new_module


def _build_bass(legalize=True):
    nc = bass.Bass()
    dt = nc.dram_tensor
    keys2t = dt("keys2t", [21, SHARD], BF16, kind="ExternalInput")
    knn_rhs = dt("knn_rhs", [21, F], BF16, kind="ExternalInput")
    db_rows = dt("db_rows", [F, ROW], BF16, kind="ExternalInput")
    feat_sh = dt("feat_sh", [C, SHARD], BF16, kind="ExternalInput")
    pcd_sh = dt("pcd_sh", [4, SHARD], BF16, kind="ExternalInput")
    pos_w1t = dt("pos_w1t", [4, PH], BF16, kind="ExternalInput")
    pos_b1 = dt("pos_b1", [PH, 1], F32, kind="ExternalInput")
    pos_w2t = dt("pos_w2t", [PH, C], BF16, kind="ExternalInput")
    pos_b2c = dt("pos_b2c", [P, 2], F32, kind="ExternalInput")
    attn_w1t = dt("attn_w1t", [C, AH], BF16, kind="ExternalInput")
    attn_b1c = dt("attn_b1c", [P, AH // P], F32, kind="ExternalInput")
    attn_w2t = dt("attn_w2t", [AH, C], BF16, kind="ExternalInput")
    out = dt("out", [C, SHARD], F32, kind="ExternalOutput")

    with tile.TileContext(nc) as tc:
        with (
            tc.tile_pool(name="const", bufs=1) as cp,
            tc.tile_pool(name="s", bufs=2) as s_pool,
            tc.tile_pool(name="idx", bufs=2) as idx_pool,
            tc.tile_pool(name="g", bufs=2) as g_pool,
            tc.tile_pool(name="h1", bufs=2) as h1_pool,
            tc.tile_pool(name="tmp", bufs=2) as tmp_pool,
            tc.tile_pool(name="small", bufs=2) as sm_pool,
            tc.tile_pool(name="ot", bufs=2) as ot_pool,
            tc.tile_pool(name="ppa", bufs=2, space="PSUM") as pp_aux,
            tc.tile_pool(name="ppm", bufs=3, space="PSUM") as pp_mlp,
        ):
            # GPSIMD ucode library with InstDMAGatherAnt (the plain-Bass
            # flow does not auto-insert library loads).
            nc.gpsimd.load_library(library_config.mlp)

            # ---- constants / weights ----
            keys2t_s = cp.tile([21, SHARD], BF16)
            nc.sync.dma_start(keys2t_s[:, :], keys2t[:, :])
            knn_rhs_s = cp.tile([21, F], BF16)
            nc.sync.dma_start(knn_rhs_s[:, :], knn_rhs[:, :])
            feat_s = []
            for cc in range(2):
                ft = cp.tile([P, SHARD], BF16, tag=f"feat{cc}")
                nc.sync.dma_start(ft[:, :], feat_sh[cc * P:(cc + 1) * P, :])
                feat_s.append(ft)
            pcd_s = cp.tile([4, SHARD], BF16)
            nc.sync.dma_start(pcd_s[:, :], pcd_sh[:, :])
            pw1 = cp.tile([4, PH], BF16)
            nc.sync.dma_start(pw1[:, :], pos_w1t[:, :])
            pb1 = cp.tile([PH, 1], F32)
            nc.sync.dma_start(pb1[:, :], pos_b1[:, :])
            pw2 = cp.tile([PH, C], BF16)
            nc.sync.dma_start(pw2[:, :], pos_w2t[:, :])
            pb2 = cp.tile([P, 2], F32)
            nc.sync.dma_start(pb2[:, :], pos_b2c[:, :])
            w1 = []
            for kc in range(2):
                wt = cp.tile([P, AH], BF16, tag=f"w1_{kc}")
                nc.sync.dma_start(wt[:, :], attn_w1t[kc * P:(kc + 1) * P, :])
                w1.append(wt)
            ab1 = cp.tile([P, AH // P], F32)
            nc.sync.dma_start(ab1[:, :], attn_b1c[:, :])
            w2 = []
            for o in range(AH // P):
                wt = cp.tile([P, C], BF16, tag=f"w2_{o}")
                nc.sync.dma_start(wt[:, :], attn_w2t[o * P:(o + 1) * P, :])
                w2.append(wt)

            g_tiles = [None] * NT

            def phase_knn(t):
                tsl = slice(t * P, (t + 1) * P)
                # KNN scores: S[p, f] = 2*k_p . f - |f|^2 (fp32)
                S = s_pool.tile([P, F], F32, name="S")
                for c in range(F // SL):
                    ps = pp_aux.tile([P, SL], F32, tag="aux", name="ks")
                    nc.tensor.matmul(ps[:, :], lhsT=keys2t_s[:, tsl],
                                     rhs=knn_rhs_s[:, c * SL:(c + 1) * SL],
                                     start=True, stop=True)
                    nc.scalar.activation(S[:, c * SL:(c + 1) * SL],
                                         ps[:, :], AF.Copy)
                # top-16 (two top-8 rounds; order within 16 is free).
                mx = sm_pool.tile([P, 8], F32, tag="mx", name="mx")
                idx16 = idx_pool.tile([P, KNN], U16, name="idx16", tag="idx16")
                nc.vector.max(out=mx[:, :], in_=S[:, :])
                nc.vector.max_index(idx16[:, 0:8], mx[:, :], S[:, :])
                nc.vector.match_replace(out=S[:, :], in_to_replace=mx[:, :],
                                        in_values=S[:, :], imm_value=NEG_BIG)
                mx2 = sm_pool.tile([P, 8], F32, tag="mx2", name="mx2")
                nc.vector.max(out=mx2[:, :], in_=S[:, :])
                nc.vector.max_index(idx16[:, 8:16], mx2[:, :], S[:, :])
                # idx -> [128, 128]: replicate 8x along free, then HWDGE
                # transpose so idxsT[16r+s, j] = idx16[j, s].
                idxw = idx_pool.tile([P, P], U16, name="idxw", tag="idxw")
                # u32-bitcast copy: bitwise-safe (bf16 would flush the
                # denormal bit patterns of indices < 128 to zero).
                nc.vector.tensor_copy(
                    idxw[:, :].bitcast(mybir.dt.uint32).rearrange(
                        "p (r s) -> p r s", r=8),
                    idx16[:, :].bitcast(mybir.dt.uint32).unsqueeze(
                        1).to_broadcast([P, 8, KNN // 2]),
                )
                idxsT = idx_pool.tile([P, P], U16, name="idxsT", tag="idxsT")
                nc.sync.dma_start_transpose(idxsT[:, :].bitcast(BF16),
                                            idxw[:, :].bitcast(BF16))
                # ONE channel-major gather of all 2048 (point, neighbor)
                # rows: G3[p, ch, j*16+k] = db[idx[j,k], 128*ch+p].
                g3 = g_pool.tile([P, 3 * P * KNN], BF16, name="g3", tag="g3")
                nc.gpsimd.dma_gather(
                    g3[:, :].rearrange("p (c i) -> p c i", c=3),
                    db_rows[:, :],
                    idxsT[:, :].bitcast(I16),
                    P * KNN,
                    P * KNN,
                    ROW,
                    transpose=True,
                    single_packet=False,
                )
                g_tiles[t] = g3

            prep_tiles = [None] * NT

            def phase_prep(t):
                tsl = slice(t * P, (t + 1) * P)
                g3 = g_tiles[t]
                G = [g3[:, :].rearrange("p (c i) -> p c i", c=3)[:, cc, :]
                     for cc in range(2)]
                P3 = g3[0:4, :].rearrange("p (c i) -> p c i", c=3)[:, 2, :]

                # ---- pos MLP (k-inner: col = j*16 + k) ----
                pr = tmp_pool.tile([4, P * KNN], BF16, tag="pr", name="pr",
                                   bufs=1)
                pcd_b = pcd_s[:, tsl].unsqueeze(2).to_broadcast([4, P, KNN])
                nc.vector.tensor_sub(
                    pr[:, :].rearrange("p (j k) -> p j k", k=KNN),
                    pcd_b, P3.rearrange("p (j k) -> p j k", k=KNN))
                h1p = tmp_pool.tile([PH, P * KNN], BF16, tag="h1p",
                                    name="h1p")
                for s in range(NSL):
                    sl = slice(s * SL, (s + 1) * SL)
                    h1p_ps = pp_aux.tile([PH, SL], F32, tag="aux", name="h1ps")
                    nc.tensor.matmul(h1p_ps[:, :], lhsT=pw1[:, :],
                                     rhs=pr[:, sl], start=True, stop=True)
                    nc.scalar.activation(h1p[:, sl], h1p_ps[:, :], AF.Relu,
                                         bias=pb1[:, 0:1])
                # ain-pre = feat_b - G (independent of pe; issued first)
                ain = [tmp_pool.tile([P, P * KNN], BF16, tag=f"ain{cc}",
                                     name=f"ain{cc}") for cc in range(2)]
                for cc in range(2):
                    featb = feat_s[cc][:, tsl].unsqueeze(2).to_broadcast(
                        [P, P, KNN])
                    nc.vector.tensor_sub(
                        ain[cc][:, :].rearrange("p (j k) -> p j k", k=KNN),
                        featb,
                        G[cc].rearrange("p (j k) -> p j k", k=KNN))
                # pe (includes pos_b2 via the activation bias)
                pe = [tmp_pool.tile([P, P * KNN], BF16, tag=f"pe{cc}",
                                    name=f"pe{cc}") for cc in range(2)]
                for cc in range(2):
                    for s in range(NSL):
                        sl = slice(s * SL, (s + 1) * SL)
                        pe_ps = pp_aux.tile([P, SL], F32, tag="aux",
                                            name="peps")
                        nc.tensor.matmul(pe_ps[:, :],
                                         lhsT=pw2[:, cc * P:(cc + 1) * P],
                                         rhs=h1p[:, sl], start=True,
                                         stop=True)
                        nc.scalar.activation(pe[cc][:, sl], pe_ps[:, :],
                                             AF.Identity,
                                             bias=pb2[:, cc:cc + 1])
                # ain += pe ; Vf = G + pe
                Vf = [tmp_pool.tile([P, P * KNN], BF16, tag=f"v{cc}",
                                    name=f"v{cc}") for cc in range(2)]
                ef = [tmp_pool.tile([P, P * KNN], BF16, tag=f"e{cc}",
                                    name=f"e{cc}", bufs=1) for cc in range(2)]
                out_t = [ot_pool.tile([P, P], F32, tag=f"ot{cc}",
                                      name=f"ot{cc}") for cc in range(2)]
                for cc in range(2):
                    nc.vector.tensor_add(ain[cc][:, :], ain[cc][:, :],
                                         pe[cc][:, :])
                    nc.vector.tensor_add(Vf[cc][:, :], pe[cc][:, :], G[cc])
                prep_tiles[t] = (ain, Vf, ef, out_t)

            def fold16(dst, src_ap, width, dt_):
                """One fold step: halve the k-extent of src_ap into dst."""
                nc.vector.tensor_add(
                    dst,
                    src_ap[:, :, 0:width // 2],
                    src_ap[:, :, width // 2:width])

            def phase_attn(t):
                tsl = slice(t * P, (t + 1) * P)
                ain, Vf, ef, out_t = prep_tiles[t]
                for sp in range(NSL // 2):
                    psl = slice(sp * 2 * SL, (sp + 1) * 2 * SL)
                    h1s = []
                    for o in range(AH // P):
                        hp = pp_mlp.tile([P, 2 * SL], F32, tag="mm2",
                                         name="hp")
                        for sh in range(2):
                            for kc in range(2):
                                nc.tensor.matmul(
                                    hp[:, sh * SL:(sh + 1) * SL],
                                    lhsT=w1[kc][:, o * P:(o + 1) * P],
                                    rhs=ain[kc][:, sp * 2 * SL + sh * SL:
                                                 sp * 2 * SL + (sh + 1) * SL],
                                    start=(kc == 0), stop=(kc == 1))
                        ht = h1_pool.tile([P, 2 * SL], BF16, tag=f"h1_{o}",
                                          name=f"h1_{o}")
                        nc.scalar.activation(ht[:, :], hp[:, :], AF.Relu,
                                             bias=ab1[:, o:o + 1])
                        h1s.append(ht)
                    for cc in range(2):
                        lp = pp_mlp.tile([P, 2 * SL], F32, tag="mm2",
                                         name="lp")
                        for o in range(AH // P):
                            for sh in range(2):
                                nc.tensor.matmul(
                                    lp[:, sh * SL:(sh + 1) * SL],
                                    lhsT=w2[o][:, cc * P:(cc + 1) * P],
                                    rhs=h1s[o][:, sh * SL:(sh + 1) * SL],
                                    start=(o == 0),
                                    stop=(o == AH // P - 1))
                        nc.scalar.activation(ef[cc][:, psl], lp[:, :], AF.Exp)
                    # per-sp softmax over the 16 k-inner neighbors + weighted
                    # sum: refined = sum(e*V)/sum(e) (pos_b2 already in V).
                    jsl = slice(sp * (P // 2), (sp + 1) * (P // 2))
                    for cc in range(2):
                        efs = ef[cc][:, psl]
                        ev = sm_pool.tile([P, 2 * SL], BF16, tag="ev",
                                          name="ev", bufs=1)
                        nc.vector.tensor_mul(ev[:, :], efs, Vf[cc][:, psl])
                        halves = []
                        for what, src in (("d", efs), ("n", ev[:, :])):
                            a8 = sm_pool.tile([P, SL], BF16, tag=f"a8{what}",
                                              name=f"a8{what}", bufs=1)
                            fold16(a8[:, :].rearrange("p (j k) -> p j k", k=8),
                                   src.rearrange("p (j k) -> p j k", k=KNN),
                                   KNN, BF16)
                            a4 = sm_pool.tile([P, SL // 2], BF16,
                                              tag=f"a4{what}",
                                              name=f"a4{what}", bufs=1)
                            fold16(a4[:, :].rearrange("p (j k) -> p j k", k=4),
                                   a8[:, :].rearrange("p (j k) -> p j k", k=8),
                                   8, BF16)
                            a2 = sm_pool.tile([P, SL // 4], BF16,
                                              tag=f"a2{what}",
                                              name=f"a2{what}", bufs=1)
                            fold16(a2[:, :].rearrange("p (j k) -> p j k", k=2),
                                   a4[:, :].rearrange("p (j k) -> p j k", k=4),
                                   4, BF16)
                            a1 = sm_pool.tile([P, SL // 8], F32,
                                              tag=f"a1{what}",
                                              name=f"a1{what}", bufs=1)
                            fold16(a1[:, :].unsqueeze(2),
                                   a2[:, :].rearrange("p (j k) -> p j k", k=2),
                                   2, F32)
                            halves.append(a1)
                        den, num = halves
                        rden = sm_pool.tile([P, SL // 8], F32, tag="rden",
                                            name="rden", bufs=1)
                        nc.vector.reciprocal(rden[:, :], den[:, :])
                        nc.vector.tensor_mul(out_t[cc][:, jsl], num[:, :],
                                             rden[:, :])
                for cc in range(2):
                    nc.sync.dma_start(out[cc * P:(cc + 1) * P, tsl],
                                      out_t[cc][:, :])

            # software pipeline: knn(t) ahead by 2, prep(t) ahead by 1,
            # so each prep's cross-engine chain overlaps the previous
            # tile's attn matmul stream.
            phase_knn(0)
            phase_knn(1)
            phase_prep(0)
            for t in range(NT):
                if t + 2 < NT:
                    phase_knn(t + 2)
                if t + 1 < NT:
                    phase_prep(t + 1)
                phase_attn(t)
    if legalize:
        _legalize_sync_waits(nc)
    # Encode bass_isa InstISA subclasses (the manual load_library pseudo) —
    # plain-Bass flow does not run Bacc's codegen_inst_isa_subclasses pass.
    mybir.codegen_inst_isa_subclasses(nc)
    return nc


_NC = None


def _get_nc():
    global _NC
    if _NC is None:
        _NC = _build_bass()
    return _NC


def _prep_in_maps(pcd, feat, pcd_feadb, feat_feadb,
                  pos_w1, pos_b1, pos_g1, pos_be1, pos_w2, pos_b2,
                  attn_w1, attn_b1, attn_g1, attn_be1, attn_w2, attn_b2):
    f32 = np.float32
    bf16 = mybir.dt.np(BF16)
    a = {k: np.ascontiguousarray(np.asarray(v), dtype=f32) for k, v in dict(
        pcd=pcd, feat=feat, pcd_feadb=pcd_feadb, feat_feadb=feat_feadb,
        pos_w1=pos_w1, pos_b1=pos_b1, pos_g1=pos_g1, pos_be1=pos_be1,
        pos_w2=pos_w2, pos_b2=pos_b2,
        attn_w1=attn_w1, attn_b1=attn_b1, attn_g1=attn_g1, attn_be1=attn_be1,
        attn_w2=attn_w2, attn_b2=attn_b2).items()}

    fus_pcd = np.concatenate([a['pcd'], a['pcd_feadb']], axis=2)    # [B,3,F]
    fus_feat = np.concatenate([a['feat'], a['feat_feadb']], axis=2)  # [B,C,F]

    # BatchNorm (eval, running stats 0/1) folded into the conv weights.
    sp = (a['pos_g1'].astype(np.float64) / np.sqrt(1.0 + BN_EPS))
    w1p = a['pos_w1'].astype(np.float64) * sp[:, None]
    b1p = a['pos_b1'].astype(np.float64) * sp + a['pos_be1']
    sa = (a['attn_g1'].astype(np.float64) / np.sqrt(1.0 + BN_EPS))
    w1a = a['attn_w1'].astype(np.float64) * sa[:, None]
    # pos_b2 is applied on-chip as the pe-copy bias (pe == reference
    # pos_embedding), so attn layer-1 bias needs no pos_b2 fold.
    b1a = a['attn_b1'].astype(np.float64) * sa + a['attn_be1']

    pos_w1t = np.zeros((4, PH), bf16)
    pos_w1t[:3] = w1p.T.astype(bf16)
    pos_b1v = b1p.astype(f32).reshape(PH, 1)
    pos_w2t = np.ascontiguousarray(a['pos_w2'].T).astype(bf16)
    pos_b2c = np.ascontiguousarray(a['pos_b2'].reshape(2, P).T)
    attn_w1t = np.ascontiguousarray(w1a.T).astype(bf16)
    attn_b1c = np.ascontiguousarray(b1a.astype(f32).reshape(AH // P, P).T)
    attn_w2t = np.ascontiguousarray(a['attn_w2'].T).astype(bf16)

    def split3(x):
        h = x.astype(bf16).astype(f32)
        r = x - h
        m = r.astype(bf16).astype(f32)
        lo = (r - m).astype(bf16).astype(f32)
        return h, m, lo

    per_batch = []
    for b in range(B):
        # 21-row bf16 split of S[f] = sum_d 2k_d f_d - |f|^2 (see v1).
        fus = fus_pcd[b].astype(f32)
        fh, fm, fl = split3(fus)                    # [3, F] each
        n64 = -np.sum(fus_pcd[b].astype(np.float64) ** 2, axis=0)
        nh, nm, nl = split3(n64.astype(f32) * 0 + n64)  # split fp64 value
        knn_rhs = np.zeros((21, F), bf16)
        r = 0
        rhs_rows = []
        for dd in range(3):
            rhs_rows += [(fh[dd], 'h', dd), (fm[dd], 'm', dd),
                         (fl[dd], 'l', dd), (fh[dd], 'H2', dd),
                         (fm[dd], 'M2', dd), (fh[dd], 'H3', dd)]
        for row, _, _ in rhs_rows:
            knn_rhs[r] = row.astype(bf16)
            r += 1
        knn_rhs[18] = nh.astype(bf16)
        knn_rhs[19] = nm.astype(bf16)
        knn_rhs[20] = nl.astype(bf16)
        db = np.zeros((F, ROW), bf16)
        db[:, :C] = fus_feat[b].T.astype(bf16)
        db[:, C:C + 3] = fus_pcd[b].T.astype(bf16)
        per_batch.append((knn_rhs, np.ascontiguousarray(db)))

    in_maps = []
    for core in range(NCORES):
        b, s = divmod(core, NCORES // B)
        sh = slice(s * SHARD, (s + 1) * SHARD)
        k2 = 2.0 * a['pcd'][b][:, sh].astype(f32)
        kh, km, kl = split3(k2)
        keys2t = np.zeros((21, SHARD), bf16)
        r = 0
        for dd in range(3):
            for krow in (kh[dd], kh[dd], kh[dd], km[dd], km[dd], kl[dd]):
                keys2t[r] = krow.astype(bf16)
                r += 1
        keys2t[18] = 1.0
        keys2t[19] = 1.0
        keys2t[20] = 1.0
        pcd_sh = np.zeros((4, SHARD), bf16)
        pcd_sh[:3] = a['pcd'][b][:, sh].astype(bf16)
        in_maps.append(dict(
            keys2t=keys2t,
            knn_rhs=per_batch[b][0],
            db_rows=per_batch[b][1],
            feat_sh=np.ascontiguousarray(a['feat'][b][:, sh]).astype(bf16),
            pcd_sh=pcd_sh,
            pos_w1t=pos_w1t, pos_b1=pos_b1v, pos_w2t=pos_w2t, pos_b2c=pos_b2c,
            attn_w1t=attn_w1t, attn_b1c=attn_b1c,
            attn_w2t=attn_w2t,
        ))
    return in_maps


def kernel(**inputs):
    global LAST_RESULT
    nc = _get_nc()
    in_maps = _prep_in_maps(**inputs)
    res = bass_utils.run_bass_kernel_spmd(
        nc, in_maps, core_ids=list(range(NCORES)), trace=TRACE)
    LAST_RESULT = res
    out = np.empty((B, C, N), np.float32)
    for core in range(NCORES):
        b, s = divmod(core, NCORES // B)
        out[b][:, s * SHARD:(s + 1) * SHARD] = res.results[core]["out"]
    return out


# revision 11
# speedup vs baseline: 1.2853x; 1.2853x over previous
"""CrossTransformer (KNN message passing) Trainium2 kernel.

Contract: kernel(**inputs) takes the FULL unsharded inputs (numpy arrays,
keys as in setup_inputs()) and returns the FULL [2, 256, 2048] float32
output.  Internally shards across 8 NeuronCores: core = b*4 + s handles
batch b, key-point shard s (512 points), with the fused KNN database
replicated per core.

Pipeline per core (v3 — split channel-major dma_gather edition):
  1. KNN scores S = 2*k.f - |f|^2 via a 21-row bf16 mantissa-split matmul
     (fp32-grade scores: selected neighbor sets match the fp32 reference);
     top-16 via DVE max/max_index/match_replace (two top-8 rounds).
  2. Index tile [128 pts, 16 slots] -> u16 -> broadcast x8 along free ->
     HWDGE dma_start_transpose -> [128, 128] (transposed + replicated
     across partition groups).  TWO SWDGE dma_gather(transpose=True) per
     tile (one per 64-point half, slicing the same index tile) pull rows
     of the [4096, 384] bf16 database (256 feat | 3 pcd | 125 zero-pad)
     landing CHANNEL-MAJOR: G3[p, ch, (j-j0)*16+k] = db[idx[j,k],
     128*ch+p].  Replaces v1's 16 indirect row-gathers + 48 PE transposes
     + 12 DVE copies per tile.  Pair layout is k-INNER within each half.
  3. prep and attn phases run per 64-point HALF so each half's gather,
     pos/attn MLPs and softmax overlap the neighbouring halves' work.
     BatchNorm folded into weights host-side; pos_b2 applied as the
     pe-copy Identity bias (pe == reference pos_embedding); attn_b2
     dropped entirely (constant along the softmax axis).
  4. Per-channel softmax over the 16 contiguous k-inner neighbors via
     bf16 fold chains, fused with the weighted sum.
"""

import copy as _copy

import numpy as np

import concourse.bass as bass
import concourse.mybir as mybir
import concourse.tile as tile
from concourse import bass_utils
from concourse import library_config

F32 = mybir.dt.float32
BF16 = mybir.dt.bfloat16
U16 = mybir.dt.uint16
U32 = mybir.dt.uint32
I16 = mybir.dt.int16
AF = mybir.ActivationFunctionType

B = 2
C = 256
N = 2048
M = 2048
F = N + M            # fused database size
KNN = 16
PH = 64              # pos MLP hidden
AH = 1024            # attn MLP hidden
P = 128
NCORES = 8
SHARD = N * B // NCORES      # 512 key points per core
NT = SHARD // P              # 4 point-tiles per core
ROW = 384                    # db row: 256 feat + 3 pcd + 125 pad (768B)
SL = 512                     # matmul free-dim slice
HJ = P // 2                  # 64 points per half
HCOL = HJ * KNN              # 1024 columns per half
BN_EPS = 1e-5
NEG_BIG = -3.0e38

# Module-level knobs for test harnesses (not used by the grader).
TRACE = False
LAST_RESULT = None


def _legalize_sync_waits(nc, max_waits=1):
    """walrus here accepts at most one sync wait per instruction; move
    extra waits onto ENGINE_NOP carriers inserted just before the offender
    (same engine: the sequencer accumulates the waits, no pipeline drain)."""
    module = nc.m
    new_module = _copy.replace(module, functions=[])
    for function in module.functions:
        new_function = _copy.replace(function, blocks=[])
        new_function.set_allocations_from_list(function.allocations)
        for block in function.blocks:
            out = []
            for inst in block.instructions:
                si = inst.sync_info
                waits = list(si.on_wait) if si is not None else []
                if len(waits) > max_waits:
                    extra, keep = waits[:-max_waits], waits[-max_waits:]
                    for j in range(0, len(extra), max_waits):
                        out.append(mybir.InstDrain(
                            name=f"I-lgl-{inst.name}-{j}",
                            engine=inst.engine,
                            ins=[], outs=[],
                            sync_info=mybir.SyncInfo(
                                on_wait=extra[j:j + max_waits], on_update=[]),
                        ))
                    inst.sync_info = mybir.SyncInfo(
                        on_wait=keep, on_update=list(si.on_update))
                out.append(inst)
            new_function.blocks.append(_copy.replace(block, instructions=out))
        new_module.functions.append(new_function)
    nc.m = new_module


def _build_bass(legalize=True):
    nc = bass.Bass()
    dt = nc.dram_tensor
    keys2t = dt("keys2t", [21, SHARD], BF16, kind="ExternalInput")
    knn_rhs = dt("knn_rhs", [21, F], BF16, kind="ExternalInput")
    db_rows = dt("db_rows", [F, ROW], BF16, kind="ExternalInput")
    feat_sh = dt("feat_sh", [C, SHARD], BF16, kind="ExternalInput")
    pcd_sh = dt("pcd_sh", [4, SHARD], BF16, kind="ExternalInput")
    pos_w1t = dt("pos_w1t", [4, PH], BF16, kind="ExternalInput")
    pos_b1 = dt("pos_b1", [PH, 1], F32, kind="ExternalInput")
    pos_w2t = dt("pos_w2t", [PH, C], BF16, kind="ExternalInput")
    pos_b2c = dt("pos_b2c", [P, 2], F32, kind="ExternalInput")
    attn_w1t = dt("attn_w1t", [C, AH], BF16, kind="ExternalInput")
    attn_b1c = dt("attn_b1c", [P, AH // P], F32, kind="ExternalInput")
    attn_w2t = dt("attn_w2t", [AH, C], BF16, kind="ExternalInput")
    out = dt("out", [C, SHARD], F32, kind="ExternalOutput")

    with tile.TileContext(nc) as tc:
        with (
            tc.tile_pool(name="const", bufs=1) as cp,
            tc.tile_pool(name="s", bufs=2) as s_pool,
            tc.tile_pool(name="idx", bufs=2) as idx_pool,
            tc.tile_pool(name="g", bufs=2) as g_pool,
            tc.tile_pool(name="h1", bufs=2) as h1_pool,
            tc.tile_pool(name="tmp", bufs=2) as tmp_pool,
            tc.tile_pool(name="small", bufs=2) as sm_pool,
            tc.tile_pool(name="ot", bufs=3) as ot_pool,
            tc.tile_pool(name="ppa", bufs=2, space="PSUM") as pp_aux,
            tc.tile_pool(name="ppm", bufs=3, space="PSUM") as pp_mlp,
        ):
            # GPSIMD ucode library with InstDMAGatherAnt (the plain-Bass
            # flow does not auto-insert library loads).
            nc.gpsimd.load_library(library_config.mlp)

            # ---- constants / weights ----
            keys2t_s = cp.tile([21, SHARD], BF16)
            nc.sync.dma_start(keys2t_s[:, :], keys2t[:, :])
            knn_rhs_s = cp.tile([21, F], BF16)
            nc.sync.dma_start(knn_rhs_s[:, :], knn_rhs[:, :])
            feat_s = []
            for cc in range(2):
                ft = cp.tile([P, SHARD], BF16, tag=f"feat{cc}")
                nc.sync.dma_start(ft[:, :], feat_sh[cc * P:(cc + 1) * P, :])
                feat_s.append(ft)
            pcd_s = cp.tile([4, SHARD], BF16)
            nc.sync.dma_start(pcd_s[:, :], pcd_sh[:, :])
            pw1 = cp.tile([4, PH], BF16)
            nc.sync.dma_start(pw1[:, :], pos_w1t[:, :])
            pb1 = cp.tile([PH, 1], F32)
            nc.sync.dma_start(pb1[:, :], pos_b1[:, :])
            pw2 = cp.tile([PH, C], BF16)
            nc.sync.dma_start(pw2[:, :], pos_w2t[:, :])
            pb2 = cp.tile([P, 2], F32)
            nc.sync.dma_start(pb2[:, :], pos_b2c[:, :])
            w1 = []
            for kc in range(2):
                wt = cp.tile([P, AH], BF16, tag=f"w1_{kc}")
                nc.sync.dma_start(wt[:, :], attn_w1t[kc * P:(kc + 1) * P, :])
                w1.append(wt)
            ab1 = cp.tile([P, AH // P], F32)
            nc.sync.dma_start(ab1[:, :], attn_b1c[:, :])
            w2 = []
            for o in range(AH // P):
                wt = cp.tile([P, C], BF16, tag=f"w2_{o}")
                nc.sync.dma_start(wt[:, :], attn_w2t[o * P:(o + 1) * P, :])
                w2.append(wt)

            idxsT_tiles = [None] * NT
            g_tiles = [[None, None] for _ in range(NT)]
            prep_tiles = [[None, None] for _ in range(NT)]
            out_tiles = [None] * NT

            def phase_knn(t):
                tsl = slice(t * P, (t + 1) * P)
                # KNN scores: S[p, f] = 2*k_p . f - |f|^2 (fp32)
                S = s_pool.tile([P, F], F32, name="S")
                for c in range(F // SL):
                    ps = pp_aux.tile([P, SL], F32, tag="aux", name="ks")
                    nc.tensor.matmul(ps[:, :], lhsT=keys2t_s[:, tsl],
                                     rhs=knn_rhs_s[:, c * SL:(c + 1) * SL],
                                     start=True, stop=True)
                    nc.scalar.activation(S[:, c * SL:(c + 1) * SL],
                                         ps[:, :], AF.Copy)
                # top-16 (two top-8 rounds; order within 16 is free).
                mx = sm_pool.tile([P, 8], F32, tag="mx", name="mx")
                idx32 = idx_pool.tile([P, KNN], U32, name="idx32", tag="idx32")
                nc.vector.max(out=mx[:, :], in_=S[:, :])
                nc.vector.max_index(idx32[:, 0:8], mx[:, :], S[:, :])
                nc.vector.match_replace(out=S[:, :], in_to_replace=mx[:, :],
                                        in_values=S[:, :], imm_value=NEG_BIG)
                mx2 = sm_pool.tile([P, 8], F32, tag="mx2", name="mx2")
                nc.vector.max(out=mx2[:, :], in_=S[:, :])
                nc.vector.max_index(idx32[:, 8:16], mx2[:, :], S[:, :])
                # u32 -> u16 (values < 4096: exact under either truncation
                # or value conversion)
                idx16 = idx_pool.tile([P, KNN], U16, name="idx16", tag="idx16")
                nc.vector.tensor_copy(idx16[:, :], idx32[:, :])
                # replicate x8 along free (u32-bitcast: bitwise-safe), then
                # HWDGE transpose so idxsT[16r+s, j] = idx16[j, s].
                idxw = idx_pool.tile([P, P], U16, name="idxw", tag="idxw")
                nc.vector.tensor_copy(
                    idxw[:, :].bitcast(U32).rearrange(
                        "p (r s) -> p r s", r=8),
                    idx16[:, :].bitcast(U32).unsqueeze(1).to_broadcast(
                        [P, 8, KNN // 2]),
                )
                idxsT = idx_pool.tile([P, P], U16, name="idxsT", tag="idxsT")
                nc.sync.dma_start_transpose(idxsT[:, :].bitcast(BF16),
                                            idxw[:, :].bitcast(BF16))
                idxsT_tiles[t] = idxsT
                out_tiles[t] = [ot_pool.tile([P, P], F32, tag=f"ot{cc}",
                                             name=f"ot{cc}")
                                for cc in range(2)]

            def phase_gather(t, h):
                # channel-major gather of one 64-point half:
                # G3[p, ch, (j-64h)*16+k] = db[idx[j, k], 128*ch+p]
                idxsT = idxsT_tiles[t]
                g3 = g_pool.tile([P, 3 * HCOL], BF16, name=f"g3{h}",
                                 tag=f"g3{h}")
                nc.gpsimd.dma_gather(
                    g3[:, :].rearrange("p (c i) -> p c i", c=3),
                    db_rows[:, :],
                    idxsT[:, h * HJ:(h + 1) * HJ].bitcast(I16),
                    HCOL,
                    HCOL,
                    ROW,
                    transpose=True,
                    single_packet=False,
                )
                g_tiles[t][h] = g3

            def phase_prep(t, h):
                jsl = slice(t * P + h * HJ, t * P + (h + 1) * HJ)
                g3 = g_tiles[t][h]
                G = [g3[:, cc * HCOL:(cc + 1) * HCOL] for cc in range(2)]
                P3 = g3[0:4, 2 * HCOL:3 * HCOL]

                # ---- pos MLP (k-inner: col = (j-64h)*16 + k) ----
                pr = tmp_pool.tile([4, HCOL], BF16, tag=f"pr{h}",
                                   name="pr", bufs=1)
                pcd_b = pcd_s[:, jsl].unsqueeze(2).to_broadcast([4, HJ, KNN])
                nc.vector.tensor_sub(
                    pr[:, :].rearrange("p (j k) -> p j k", k=KNN),
                    pcd_b, P3.rearrange("p (j k) -> p j k", k=KNN))
                h1p = tmp_pool.tile([PH, HCOL], BF16, tag=f"h1p{h}",
                                    name="h1p")
                for s in range(HCOL // SL):
                    sl = slice(s * SL, (s + 1) * SL)
                    h1p_ps = pp_aux.tile([PH, SL], F32, tag="aux", name="h1ps")
                    nc.tensor.matmul(h1p_ps[:, :], lhsT=pw1[:, :],
                                     rhs=pr[:, sl], start=True, stop=True)
                    nc.scalar.activation(h1p[:, sl], h1p_ps[:, :], AF.Relu,
                                         bias=pb1[:, 0:1])
                # ain-pre = feat_b - G (independent of pe; issued first)
                ain = [tmp_pool.tile([P, HCOL], BF16, tag=f"ain{cc}{h}",
                                     name=f"ain{cc}") for cc in range(2)]
                for cc in range(2):
                    featb = feat_s[cc][:, jsl].unsqueeze(2).to_broadcast(
                        [P, HJ, KNN])
                    nc.vector.tensor_sub(
                        ain[cc][:, :].rearrange("p (j k) -> p j k", k=KNN),
                        featb,
                        G[cc].rearrange("p (j k) -> p j k", k=KNN))
                # pe (includes pos_b2 via the activation bias)
                pe = [tmp_pool.tile([P, HCOL], BF16, tag=f"pe{cc}{h}",
                                    name=f"pe{cc}") for cc in range(2)]
                for cc in range(2):
                    for s in range(HCOL // SL):
                        sl = slice(s * SL, (s + 1) * SL)
                        pe_ps = pp_aux.tile([P, SL], F32, tag="aux",
                                            name="peps")
                        nc.tensor.matmul(pe_ps[:, :],
                                         lhsT=pw2[:, cc * P:(cc + 1) * P],
                                         rhs=h1p[:, sl], start=True,
                                         stop=True)
                        nc.scalar.activation(pe[cc][:, sl], pe_ps[:, :],
                                             AF.Identity,
                                             bias=pb2[:, cc:cc + 1])
                # ain += pe ; Vf = G + pe
                Vf = [tmp_pool.tile([P, HCOL], BF16, tag=f"v{cc}{h}",
                                    name=f"v{cc}") for cc in range(2)]
                for cc in range(2):
                    nc.vector.tensor_add(ain[cc][:, :], ain[cc][:, :],
                                         pe[cc][:, :])
                    nc.vector.tensor_add(Vf[cc][:, :], pe[cc][:, :], G[cc])
                prep_tiles[t][h] = (ain, Vf)

            def phase_attn(t, h):
                tsl = slice(t * P, (t + 1) * P)
                ain, Vf = prep_tiles[t][h]
                out_t = out_tiles[t]
                h1s = []
                for o in range(AH // P):
                    hp = pp_mlp.tile([P, 2 * SL], F32, tag="mm2", name="hp")
                    # kc-outer: stationary w1[kc] column loaded once per
                    # pair of sh halves
                    for kc in range(2):
                        for sh in range(2):
                            nc.tensor.matmul(
                                hp[:, sh * SL:(sh + 1) * SL],
                                lhsT=w1[kc][:, o * P:(o + 1) * P],
                                rhs=ain[kc][:, sh * SL:(sh + 1) * SL],
                                start=(kc == 0), stop=(kc == 1))
                    ht = h1_pool.tile([P, 2 * SL], BF16, tag=f"h1_{o}",
                                      name=f"h1_{o}")
                    nc.scalar.activation(ht[:, :], hp[:, :], AF.Relu,
                                         bias=ab1[:, o:o + 1])
                    h1s.append(ht)
                jsl = slice(h * HJ, (h + 1) * HJ)
                for cc in range(2):
                    lp = pp_mlp.tile([P, 2 * SL], F32, tag="mm2", name="lp")
                    for o in range(AH // P):
                        for sh in range(2):
                            nc.tensor.matmul(
                                lp[:, sh * SL:(sh + 1) * SL],
                                lhsT=w2[o][:, cc * P:(cc + 1) * P],
                                rhs=h1s[o][:, sh * SL:(sh + 1) * SL],
                                start=(o == 0),
                                stop=(o == AH // P - 1))
                    ef = sm_pool.tile([P, 2 * SL], BF16, tag=f"ef{cc}",
                                      name=f"ef{cc}")
                    nc.scalar.activation(ef[:, :], lp[:, :], AF.Exp)
                    # softmax over the 16 k-inner neighbors + weighted sum:
                    # refined = sum(e*V)/sum(e) (pos_b2 already inside V).
                    ev = sm_pool.tile([P, 2 * SL], BF16, tag=f"ev{cc}",
                                      name="ev", bufs=1)
                    nc.vector.tensor_mul(ev[:, :], ef[:, :], Vf[cc][:, :])
                    halves = []
                    for what, src in (("d", ef), ("n", ev)):
                        a8 = sm_pool.tile([P, SL], BF16, tag=f"a8{what}",
                                          name=f"a8{what}", bufs=1)
                        nc.vector.tensor_add(
                            a8[:, :].rearrange("p (j k) -> p j k", k=8),
                            src[:, :].rearrange(
                                "p (j k) -> p j k", k=KNN)[:, :, 0:8],
                            src[:, :].rearrange(
                                "p (j k) -> p j k", k=KNN)[:, :, 8:16])
                        a4 = sm_pool.tile([P, SL // 2], BF16, tag=f"a4{what}",
                                          name=f"a4{what}", bufs=1)
                        nc.vector.tensor_add(
                            a4[:, :].rearrange("p (j k) -> p j k", k=4),
                            a8[:, :].rearrange(
                                "p (j k) -> p j k", k=8)[:, :, 0:4],
                            a8[:, :].rearrange(
                                "p (j k) -> p j k", k=8)[:, :, 4:8])
                        a2 = sm_pool.tile([P, SL // 4], BF16, tag=f"a2{what}",
                                          name=f"a2{what}", bufs=1)
                        nc.vector.tensor_add(
                            a2[:, :].rearrange("p (j k) -> p j k", k=2),
                            a4[:, :].rearrange(
                                "p (j k) -> p j k", k=4)[:, :, 0:2],
                            a4[:, :].rearrange(
                                "p (j k) -> p j k", k=4)[:, :, 2:4])
                        a1 = sm_pool.tile([P, SL // 8], F32, tag=f"a1{what}",
                                          name=f"a1{what}", bufs=1)
                        nc.vector.tensor_add(
                            a1[:, :].unsqueeze(2),
                            a2[:, :].rearrange(
                                "p (j k) -> p j k", k=2)[:, :, 0:1],
                            a2[:, :].rearrange(
                                "p (j k) -> p j k", k=2)[:, :, 1:2])
                        halves.append(a1)
                    den, num = halves
                    rden = sm_pool.tile([P, SL // 8], F32, tag="rden",
                                        name="rden", bufs=1)
                    nc.vector.reciprocal(rden[:, :], den[:, :])
                    nc.vector.tensor_mul(out_t[cc][:, jsl], num[:, :],
                                         rden[:, :])
                if h == 1:
                    for cc in range(2):
                        nc.sync.dma_start(out[cc * P:(cc + 1) * P, tsl],
                                          out_t[cc][:, :])

            # software pipeline over 8 half-tile jobs; knn runs ~2 tiles
            # ahead, each half's gather+prep runs one job ahead of its attn.
            phase_knn(0)
            phase_gather(0, 0)
            phase_prep(0, 0)
            phase_knn(1)
            phase_gather(0, 1)
            phase_prep(0, 1)
            for t in range(NT):
                phase_attn(t, 0)
                if t + 2 < NT:
                    phase_knn(t + 2)
                if t + 1 < NT:
                    phase_gather(t + 1, 0)
                    phase_prep(t + 1, 0)
                phase_attn(t, 1)
                if t + 1 < NT:
                    phase_gather(t + 1, 1)
                    phase_prep(t + 1, 1)
    if legalize:
        _legalize_sync_waits(nc)
    # Encode bass_isa InstISA subclasses (the manual load_library pseudo) —
    # plain-Bass flow does not run Bacc's codegen_inst_isa_subclasses pass.
    mybir.codegen_inst_isa_subclasses(nc)
    return nc


_NC = None


def _get_nc():
    global _NC
    if _NC is None:
        _NC = _build_bass()
    return _NC


def _prep_in_maps(pcd, feat, pcd_feadb, feat_feadb,
                  pos_w1, pos_b1, pos_g1, pos_be1, pos_w2, pos_b2,
                  attn_w1, attn_b1, attn_g1, attn_be1, attn_w2, attn_b2):
    f32 = np.float32
    bf16 = mybir.dt.np(BF16)
    a = {k: np.ascontiguousarray(np.asarray(v), dtype=f32) for k, v in dict(
        pcd=pcd, feat=feat, pcd_feadb=pcd_feadb, feat_feadb=feat_feadb,
        pos_w1=pos_w1, pos_b1=pos_b1, pos_g1=pos_g1, pos_be1=pos_be1,
        pos_w2=pos_w2, pos_b2=pos_b2,
        attn_w1=attn_w1, attn_b1=attn_b1, attn_g1=attn_g1, attn_be1=attn_be1,
        attn_w2=attn_w2, attn_b2=attn_b2).items()}

    fus_pcd = np.concatenate([a['pcd'], a['pcd_feadb']], axis=2)    # [B,3,F]
    fus_feat = np.concatenate([a['feat'], a['feat_feadb']], axis=2)  # [B,C,F]

    # BatchNorm (eval, running stats 0/1) folded into the conv weights.
    sp = (a['pos_g1'].astype(np.float64) / np.sqrt(1.0 + BN_EPS))
    w1p = a['pos_w1'].astype(np.float64) * sp[:, None]
    b1p = a['pos_b1'].astype(np.float64) * sp + a['pos_be1']
    sa = (a['attn_g1'].astype(np.float64) / np.sqrt(1.0 + BN_EPS))
    w1a = a['attn_w1'].astype(np.float64) * sa[:, None]
    # pos_b2 is applied on-chip as the pe-copy bias (pe == reference
    # pos_embedding); attn_b2 is dropped (softmax-invariant).
    b1a = a['attn_b1'].astype(np.float64) * sa + a['attn_be1']

    pos_w1t = np.zeros((4, PH), bf16)
    pos_w1t[:3] = w1p.T.astype(bf16)
    pos_b1v = b1p.astype(f32).reshape(PH, 1)
    pos_w2t = np.ascontiguousarray(a['pos_w2'].T).astype(bf16)
    pos_b2c = np.ascontiguousarray(a['pos_b2'].reshape(2, P).T)
    attn_w1t = np.ascontiguousarray(w1a.T).astype(bf16)
    attn_b1c = np.ascontiguousarray(b1a.astype(f32).reshape(AH // P, P).T)
    attn_w2t = np.ascontiguousarray(a['attn_w2'].T).astype(bf16)

    def split3(x):
        h = x.astype(bf16).astype(f32)
        r = x - h
        m = r.astype(bf16).astype(f32)
        lo = (r - m).astype(bf16).astype(f32)
        return h, m, lo

    per_batch = []
    for b in range(B):
        # 21-row bf16 split of S[f] = sum_d 2k_d f_d - |f|^2 (see v1).
        fus = fus_pcd[b].astype(f32)
        fh, fm, fl = split3(fus)                    # [3, F] each
        n64 = -np.sum(fus_pcd[b].astype(np.float64) ** 2, axis=0)
        nh, nm, nl = split3(n64.astype(f32) * 0 + n64)  # split fp64 value
        knn_rhs = np.zeros((21, F), bf16)
        r = 0
        rhs_rows = []
        for dd in range(3):
            rhs_rows += [(fh[dd], 'h', dd), (fm[dd], 'm', dd),
                         (fl[dd], 'l', dd), (fh[dd], 'H2', dd),
                         (fm[dd], 'M2', dd), (fh[dd], 'H3', dd)]
        for row, _, _ in rhs_rows:
            knn_rhs[r] = row.astype(bf16)
            r += 1
        knn_rhs[18] = nh.astype(bf16)
        knn_rhs[19] = nm.astype(bf16)
        knn_rhs[20] = nl.astype(bf16)
        db = np.zeros((F, ROW), bf16)
        db[:, :C] = fus_feat[b].T.astype(bf16)
        db[:, C:C + 3] = fus_pcd[b].T.astype(bf16)
        per_batch.append((knn_rhs, np.ascontiguousarray(db)))

    in_maps = []
    for core in range(NCORES):
        b, s = divmod(core, NCORES // B)
        sh = slice(s * SHARD, (s + 1) * SHARD)
        k2 = 2.0 * a['pcd'][b][:, sh].astype(f32)
        kh, km, kl = split3(k2)
        keys2t = np.zeros((21, SHARD), bf16)
        r = 0
        for dd in range(3):
            for krow in (kh[dd], kh[dd], kh[dd], km[dd], km[dd], kl[dd]):
                keys2t[r] = krow.astype(bf16)
                r += 1
        keys2t[18] = 1.0
        keys2t[19] = 1.0
        keys2t[20] = 1.0
        pcd_sh = np.zeros((4, SHARD), bf16)
        pcd_sh[:3] = a['pcd'][b][:, sh].astype(bf16)
        in_maps.append(dict(
            keys2t=keys2t,
            knn_rhs=per_batch[b][0],
            db_rows=per_batch[b][1],
            feat_sh=np.ascontiguousarray(a['feat'][b][:, sh]).astype(bf16),
            pcd_sh=pcd_sh,
            pos_w1t=pos_w1t, pos_b1=pos_b1v, pos_w2t=pos_w2t, pos_b2c=pos_b2c,
            attn_w1t=attn_w1t, attn_b1c=attn_b1c,
            attn_w2t=attn_w2t,
        ))
    return in_maps


def kernel(**inputs):
    global LAST_RESULT
    nc = _get_nc()
    in_maps = _prep_in_maps(**inputs)
    res = bass_utils.run_bass_kernel_spmd(
        nc, in_maps, core_ids=list(range(NCORES)), trace=TRACE)
    LAST_RESULT = res
    out = np.empty((B, C, N), np.float32)
    for core in range(NCORES):
        b, s = divmod(core, NCORES // B)
        out[b][:, s * SHARD:(s + 1) * SHARD] = res.results[core]["out"]
    return out


# revision 12
# speedup vs baseline: 1.3532x; 1.0528x over previous
"""CrossTransformer (KNN message passing) Trainium2 kernel.

Contract: kernel(**inputs) takes the FULL unsharded inputs (numpy arrays,
keys as in setup_inputs()) and returns the FULL [2, 256, 2048] float32
output.  Internally shards across 8 NeuronCores: core = b*4 + s handles
batch b, key-point shard s (512 points), with the fused KNN database
replicated per core.

Pipeline per core (v3 — split channel-major dma_gather edition):
  1. KNN scores S = 2*k.f - |f|^2 via a 21-row bf16 mantissa-split matmul
     (fp32-grade scores: selected neighbor sets match the fp32 reference);
     top-16 via DVE max/max_index/match_replace (two top-8 rounds).
  2. Index tile [128 pts, 16 slots] -> u16 -> broadcast x8 along free ->
     HWDGE dma_start_transpose -> [128, 128] (transposed + replicated
     across partition groups).  TWO SWDGE dma_gather(transpose=True) per
     tile (one per 64-point half, slicing the same index tile) pull rows
     of the [4096, 384] bf16 database (256 feat | 3 pcd | 125 zero-pad)
     landing CHANNEL-MAJOR: G3[p, ch, (j-j0)*16+k] = db[idx[j,k],
     128*ch+p].  Replaces v1's 16 indirect row-gathers + 48 PE transposes
     + 12 DVE copies per tile.  Pair layout is k-INNER within each half.
  3. prep and attn phases run per 64-point HALF so each half's gather,
     pos/attn MLPs and softmax overlap the neighbouring halves' work.
     BatchNorm folded into weights host-side; pos_b2 applied as the
     pe-copy Identity bias (pe == reference pos_embedding); attn_b2
     dropped entirely (constant along the softmax axis).
  4. Per-channel softmax over the 16 contiguous k-inner neighbors via
     bf16 fold chains, fused with the weighted sum.
"""

import copy as _copy

import numpy as np

import concourse.bass as bass
import concourse.mybir as mybir
import concourse.tile as tile
from concourse import bass_utils
from concourse import library_config

F32 = mybir.dt.float32
BF16 = mybir.dt.bfloat16
U16 = mybir.dt.uint16
U32 = mybir.dt.uint32
I16 = mybir.dt.int16
AF = mybir.ActivationFunctionType

B = 2
C = 256
N = 2048
M = 2048
F = N + M            # fused database size
KNN = 16
PH = 64              # pos MLP hidden
AH = 1024            # attn MLP hidden
P = 128
NCORES = 8
SHARD = N * B // NCORES      # 512 key points per core
NT = SHARD // P              # 4 point-tiles per core
ROW = 384                    # db row: 256 feat + 3 pcd + 125 pad (768B)
SL = 512                     # matmul free-dim slice
HJ = P // 2                  # 64 points per half
HCOL = HJ * KNN              # 1024 columns per half
BN_EPS = 1e-5
NEG_BIG = -3.0e38

# Module-level knobs for test harnesses (not used by the grader).
TRACE = False
LAST_RESULT = None


def _legalize_sync_waits(nc, max_waits=1):
    """walrus here accepts at most one sync wait per instruction; move
    extra waits onto ENGINE_NOP carriers inserted just before the offender
    (same engine: the sequencer accumulates the waits, no pipeline drain)."""
    module = nc.m
    new_module = _copy.replace(module, functions=[])
    for function in module.functions:
        new_function = _copy.replace(function, blocks=[])
        new_function.set_allocations_from_list(function.allocations)
        for block in function.blocks:
            out = []
            for inst in block.instructions:
                si = inst.sync_info
                waits = list(si.on_wait) if si is not None else []
                if len(waits) > max_waits:
                    extra, keep = waits[:-max_waits], waits[-max_waits:]
                    for j in range(0, len(extra), max_waits):
                        out.append(mybir.InstDrain(
                            name=f"I-lgl-{inst.name}-{j}",
                            engine=inst.engine,
                            ins=[], outs=[],
                            sync_info=mybir.SyncInfo(
                                on_wait=extra[j:j + max_waits], on_update=[]),
                        ))
                    inst.sync_info = mybir.SyncInfo(
                        on_wait=keep, on_update=list(si.on_update))
                out.append(inst)
            new_function.blocks.append(_copy.replace(block, instructions=out))
        new_module.functions.append(new_function)
    nc.m = new_module


def _build_bass(legalize=True):
    nc = bass.Bass()
    dt = nc.dram_tensor
    keys2t = dt("keys2t", [21, SHARD], BF16, kind="ExternalInput")
    knn_rhs = dt("knn_rhs", [21, F], BF16, kind="ExternalInput")
    db_rows = dt("db_rows", [F, ROW], BF16, kind="ExternalInput")
    feat_sh = dt("feat_sh", [C, SHARD], BF16, kind="ExternalInput")
    pcd_sh = dt("pcd_sh", [4, SHARD], BF16, kind="ExternalInput")
    pos_w1t = dt("pos_w1t", [4, PH], BF16, kind="ExternalInput")
    pos_b1 = dt("pos_b1", [PH, 1], F32, kind="ExternalInput")
    pos_w2t = dt("pos_w2t", [PH, C], BF16, kind="ExternalInput")
    pos_b2c = dt("pos_b2c", [P, 2], F32, kind="ExternalInput")
    attn_w1t = dt("attn_w1t", [C, AH], BF16, kind="ExternalInput")
    attn_b1c = dt("attn_b1c", [P, AH // P], F32, kind="ExternalInput")
    attn_w2t = dt("attn_w2t", [AH, C], BF16, kind="ExternalInput")
    out = dt("out", [C, SHARD], F32, kind="ExternalOutput")

    with tile.TileContext(nc) as tc:
        with (
            tc.tile_pool(name="const", bufs=1) as cp,
            tc.tile_pool(name="s", bufs=2) as s_pool,
            tc.tile_pool(name="idx", bufs=2) as idx_pool,
            tc.tile_pool(name="g", bufs=2) as g_pool,
            tc.tile_pool(name="h1", bufs=2) as h1_pool,
            tc.tile_pool(name="tmp", bufs=2) as tmp_pool,
            tc.tile_pool(name="small", bufs=2) as sm_pool,
            tc.tile_pool(name="ot", bufs=3) as ot_pool,
            tc.tile_pool(name="ppa", bufs=2, space="PSUM") as pp_aux,
            tc.tile_pool(name="ppm", bufs=3, space="PSUM") as pp_mlp,
        ):
            # GPSIMD ucode library with InstDMAGatherAnt (the plain-Bass
            # flow does not auto-insert library loads).
            nc.gpsimd.load_library(library_config.mlp)

            # ---- constants / weights ----
            keys2t_s = cp.tile([21, SHARD], BF16)
            nc.sync.dma_start(keys2t_s[:, :], keys2t[:, :])
            knn_rhs_s = cp.tile([21, F], BF16)
            nc.sync.dma_start(knn_rhs_s[:, :], knn_rhs[:, :])
            feat_s = []
            for cc in range(2):
                ft = cp.tile([P, SHARD], BF16, tag=f"feat{cc}")
                nc.sync.dma_start(ft[:, :], feat_sh[cc * P:(cc + 1) * P, :])
                feat_s.append(ft)
            pcd_s = cp.tile([4, SHARD], BF16)
            nc.sync.dma_start(pcd_s[:, :], pcd_sh[:, :])
            pw1 = cp.tile([4, PH], BF16)
            nc.sync.dma_start(pw1[:, :], pos_w1t[:, :])
            pb1 = cp.tile([PH, 1], F32)
            nc.sync.dma_start(pb1[:, :], pos_b1[:, :])
            pw2 = cp.tile([PH, C], BF16)
            nc.sync.dma_start(pw2[:, :], pos_w2t[:, :])
            pb2 = cp.tile([P, 2], F32)
            nc.sync.dma_start(pb2[:, :], pos_b2c[:, :])
            w1 = []
            for kc in range(2):
                wt = cp.tile([P, AH], BF16, tag=f"w1_{kc}")
                nc.sync.dma_start(wt[:, :], attn_w1t[kc * P:(kc + 1) * P, :])
                w1.append(wt)
            ab1 = cp.tile([P, AH // P], F32)
            nc.sync.dma_start(ab1[:, :], attn_b1c[:, :])
            w2 = []
            for o in range(AH // P):
                wt = cp.tile([P, C], BF16, tag=f"w2_{o}")
                nc.sync.dma_start(wt[:, :], attn_w2t[o * P:(o + 1) * P, :])
                w2.append(wt)

            idxsT_tiles = [None] * NT
            g_tiles = [[None, None] for _ in range(NT)]
            prep_tiles = [[None, None] for _ in range(NT)]
            out_tiles = [None] * NT

            def phase_knn(t):
                tsl = slice(t * P, (t + 1) * P)
                # KNN scores: S[p, f] = 2*k_p . f - |f|^2 (fp32)
                S = s_pool.tile([P, F], F32, name="S")
                for c in range(F // SL):
                    ps = pp_aux.tile([P, SL], F32, tag="aux", name="ks")
                    nc.tensor.matmul(ps[:, :], lhsT=keys2t_s[:, tsl],
                                     rhs=knn_rhs_s[:, c * SL:(c + 1) * SL],
                                     start=True, stop=True)
                    nc.scalar.activation(S[:, c * SL:(c + 1) * SL],
                                         ps[:, :], AF.Copy)
                # top-16 (two top-8 rounds; order within 16 is free).
                mx = sm_pool.tile([P, 8], F32, tag="mx", name="mx")
                idx32 = idx_pool.tile([P, KNN], U32, name="idx32", tag="idx32")
                nc.vector.max(out=mx[:, :], in_=S[:, :])
                nc.vector.max_index(idx32[:, 0:8], mx[:, :], S[:, :])
                nc.vector.match_replace(out=S[:, :], in_to_replace=mx[:, :],
                                        in_values=S[:, :], imm_value=NEG_BIG)
                mx2 = sm_pool.tile([P, 8], F32, tag="mx2", name="mx2")
                nc.vector.max(out=mx2[:, :], in_=S[:, :])
                nc.vector.max_index(idx32[:, 8:16], mx2[:, :], S[:, :])
                # u32 -> u16 (values < 4096: exact under either truncation
                # or value conversion)
                idx16 = idx_pool.tile([P, KNN], U16, name="idx16", tag="idx16")
                nc.vector.tensor_copy(idx16[:, :], idx32[:, :])
                # replicate x8 along free (u32-bitcast: bitwise-safe), then
                # HWDGE transpose so idxsT[16r+s, j] = idx16[j, s].
                idxw = idx_pool.tile([P, P], U16, name="idxw", tag="idxw")
                nc.vector.tensor_copy(
                    idxw[:, :].bitcast(U32).rearrange(
                        "p (r s) -> p r s", r=8),
                    idx16[:, :].bitcast(U32).unsqueeze(1).to_broadcast(
                        [P, 8, KNN // 2]),
                )
                idxsT = idx_pool.tile([P, P], U16, name="idxsT", tag="idxsT")
                nc.sync.dma_start_transpose(idxsT[:, :].bitcast(BF16),
                                            idxw[:, :].bitcast(BF16))
                idxsT_tiles[t] = idxsT
                out_tiles[t] = [ot_pool.tile([P, P], F32, tag=f"ot{cc}",
                                             name=f"ot{cc}")
                                for cc in range(2)]

            def phase_gather(t, h):
                # channel-major gather of one 64-point half:
                # G3[p, ch, (j-64h)*16+k] = db[idx[j, k], 128*ch+p]
                idxsT = idxsT_tiles[t]
                g3 = g_pool.tile([P, 3 * HCOL], BF16, name=f"g3{h}",
                                 tag=f"g3{h}")
                nc.gpsimd.dma_gather(
                    g3[:, :].rearrange("p (c i) -> p c i", c=3),
                    db_rows[:, :],
                    idxsT[:, h * HJ:(h + 1) * HJ].bitcast(I16),
                    HCOL,
                    HCOL,
                    ROW,
                    transpose=True,
                    single_packet=False,
                )
                g_tiles[t][h] = g3

            def phase_prep(t, h):
                jsl = slice(t * P + h * HJ, t * P + (h + 1) * HJ)
                g3 = g_tiles[t][h]
                G = [g3[:, cc * HCOL:(cc + 1) * HCOL] for cc in range(2)]
                P3 = g3[0:4, 2 * HCOL:3 * HCOL]

                # ---- pos MLP (k-inner: col = (j-64h)*16 + k) ----
                pr = tmp_pool.tile([4, HCOL], BF16, tag=f"pr{h}",
                                   name="pr", bufs=1)
                pcd_b = pcd_s[:, jsl].unsqueeze(2).to_broadcast([4, HJ, KNN])
                nc.vector.tensor_sub(
                    pr[:, :].rearrange("p (j k) -> p j k", k=KNN),
                    pcd_b, P3.rearrange("p (j k) -> p j k", k=KNN))
                h1p = tmp_pool.tile([PH, HCOL], BF16, tag=f"h1p{h}",
                                    name="h1p")
                for s in range(HCOL // SL):
                    sl = slice(s * SL, (s + 1) * SL)
                    h1p_ps = pp_aux.tile([PH, SL], F32, tag="aux", name="h1ps")
                    nc.tensor.matmul(h1p_ps[:, :], lhsT=pw1[:, :],
                                     rhs=pr[:, sl], start=True, stop=True)
                    nc.scalar.activation(h1p[:, sl], h1p_ps[:, :], AF.Relu,
                                         bias=pb1[:, 0:1])
                # ain-pre = feat_b - G (independent of pe; issued first)
                ain = [tmp_pool.tile([P, HCOL], BF16, tag=f"ain{cc}{h}",
                                     name=f"ain{cc}") for cc in range(2)]
                for cc in range(2):
                    featb = feat_s[cc][:, jsl].unsqueeze(2).to_broadcast(
                        [P, HJ, KNN])
                    nc.vector.tensor_sub(
                        ain[cc][:, :].rearrange("p (j k) -> p j k", k=KNN),
                        featb,
                        G[cc].rearrange("p (j k) -> p j k", k=KNN))
                # pe (includes pos_b2 via the activation bias)
                pe = [tmp_pool.tile([P, HCOL], BF16, tag=f"pe{cc}{h}",
                                    name=f"pe{cc}") for cc in range(2)]
                for cc in range(2):
                    for s in range(HCOL // SL):
                        sl = slice(s * SL, (s + 1) * SL)
                        pe_ps = pp_aux.tile([P, SL], F32, tag="aux",
                                            name="peps")
                        nc.tensor.matmul(pe_ps[:, :],
                                         lhsT=pw2[:, cc * P:(cc + 1) * P],
                                         rhs=h1p[:, sl], start=True,
                                         stop=True)
                        nc.scalar.activation(pe[cc][:, sl], pe_ps[:, :],
                                             AF.Identity,
                                             bias=pb2[:, cc:cc + 1])
                # ain += pe ; Vf = G + pe
                Vf = [tmp_pool.tile([P, HCOL], BF16, tag=f"v{cc}{h}",
                                    name=f"v{cc}") for cc in range(2)]
                for cc in range(2):
                    nc.vector.tensor_add(ain[cc][:, :], ain[cc][:, :],
                                         pe[cc][:, :])
                    nc.vector.tensor_add(Vf[cc][:, :], pe[cc][:, :], G[cc])
                prep_tiles[t][h] = (ain, Vf)

            def phase_attn(t, h):
                tsl = slice(t * P, (t + 1) * P)
                ain, Vf = prep_tiles[t][h]
                out_t = out_tiles[t]
                h1s = []
                for o in range(AH // P):
                    hp = pp_mlp.tile([P, 2 * SL], F32, tag="mm2", name="hp")
                    # kc-outer: stationary w1[kc] column loaded once per
                    # pair of sh halves
                    for kc in range(2):
                        for sh in range(2):
                            nc.tensor.matmul(
                                hp[:, sh * SL:(sh + 1) * SL],
                                lhsT=w1[kc][:, o * P:(o + 1) * P],
                                rhs=ain[kc][:, sh * SL:(sh + 1) * SL],
                                start=(kc == 0), stop=(kc == 1))
                    ht = h1_pool.tile([P, 2 * SL], BF16, tag=f"h1_{o}",
                                      name=f"h1_{o}")
                    nc.scalar.activation(ht[:, :], hp[:, :], AF.Relu,
                                         bias=ab1[:, o:o + 1])
                    h1s.append(ht)
                jsl = slice(h * HJ, (h + 1) * HJ)
                for cc in range(2):
                    lp = pp_mlp.tile([P, 2 * SL], F32, tag="mm2", name="lp")
                    for o in range(AH // P):
                        for sh in range(2):
                            nc.tensor.matmul(
                                lp[:, sh * SL:(sh + 1) * SL],
                                lhsT=w2[o][:, cc * P:(cc + 1) * P],
                                rhs=h1s[o][:, sh * SL:(sh + 1) * SL],
                                start=(o == 0),
                                stop=(o == AH // P - 1))
                    ef = sm_pool.tile([P, 2 * SL], BF16, tag=f"ef{cc}",
                                      name=f"ef{cc}")
                    nc.scalar.activation(ef[:, :], lp[:, :], AF.Exp)
                    # softmax over the 16 k-inner neighbors + weighted sum:
                    # refined = sum(e*V)/sum(e) (pos_b2 already inside V).
                    ev = sm_pool.tile([P, 2 * SL], BF16, tag=f"ev{cc}",
                                      name="ev", bufs=1)
                    nc.vector.tensor_mul(ev[:, :], ef[:, :], Vf[cc][:, :])
                    halves = []
                    for what, src in (("d", ef), ("n", ev)):
                        a8 = sm_pool.tile([P, SL], BF16, tag=f"a8{what}",
                                          name=f"a8{what}", bufs=1)
                        nc.vector.tensor_add(
                            a8[:, :].rearrange("p (j k) -> p j k", k=8),
                            src[:, :].rearrange(
                                "p (j k) -> p j k", k=KNN)[:, :, 0:8],
                            src[:, :].rearrange(
                                "p (j k) -> p j k", k=KNN)[:, :, 8:16])
                        a4 = sm_pool.tile([P, SL // 2], BF16, tag=f"a4{what}",
                                          name=f"a4{what}", bufs=1)
                        nc.vector.tensor_add(
                            a4[:, :].rearrange("p (j k) -> p j k", k=4),
                            a8[:, :].rearrange(
                                "p (j k) -> p j k", k=8)[:, :, 0:4],
                            a8[:, :].rearrange(
                                "p (j k) -> p j k", k=8)[:, :, 4:8])
                        a2 = sm_pool.tile([P, SL // 4], BF16, tag=f"a2{what}",
                                          name=f"a2{what}", bufs=1)
                        nc.vector.tensor_add(
                            a2[:, :].rearrange("p (j k) -> p j k", k=2),
                            a4[:, :].rearrange(
                                "p (j k) -> p j k", k=4)[:, :, 0:2],
                            a4[:, :].rearrange(
                                "p (j k) -> p j k", k=4)[:, :, 2:4])
                        a1 = sm_pool.tile([P, SL // 8], F32, tag=f"a1{what}",
                                          name=f"a1{what}", bufs=1)
                        nc.vector.tensor_add(
                            a1[:, :].unsqueeze(2),
                            a2[:, :].rearrange(
                                "p (j k) -> p j k", k=2)[:, :, 0:1],
                            a2[:, :].rearrange(
                                "p (j k) -> p j k", k=2)[:, :, 1:2])
                        halves.append(a1)
                    den, num = halves
                    rden = sm_pool.tile([P, SL // 8], F32, tag="rden",
                                        name="rden", bufs=1)
                    nc.vector.reciprocal(rden[:, :], den[:, :])
                    nc.vector.tensor_mul(out_t[cc][:, jsl], num[:, :],
                                         rden[:, :])
                if h == 1:
                    for cc in range(2):
                        nc.sync.dma_start(out[cc * P:(cc + 1) * P, tsl],
                                          out_t[cc][:, :])

            # software pipeline over 8 half-tile jobs; knn runs ~2 tiles
            # ahead, each half's gather+prep runs one job ahead of its attn.
            # Both head knn phases precede prep(0,0) so the DVE FIFO never
            # head-of-line blocks on gather data.
            phase_knn(0)
            phase_knn(1)
            phase_gather(0, 0)
            phase_gather(0, 1)
            phase_prep(0, 0)
            phase_prep(0, 1)
            for t in range(NT):
                phase_attn(t, 0)
                if t + 2 < NT:
                    phase_knn(t + 2)
                if t + 1 < NT:
                    phase_gather(t + 1, 0)
                    phase_prep(t + 1, 0)
                phase_attn(t, 1)
                if t + 1 < NT:
                    phase_gather(t + 1, 1)
                    phase_prep(t + 1, 1)
    if legalize:
        _legalize_sync_waits(nc)
    # Encode bass_isa InstISA subclasses (the manual load_library pseudo) —
    # plain-Bass flow does not run Bacc's codegen_inst_isa_subclasses pass.
    mybir.codegen_inst_isa_subclasses(nc)
    return nc


_NC = None


def _get_nc():
    global _NC
    if _NC is None:
        _NC = _build_bass()
    return _NC


def _prep_in_maps(pcd, feat, pcd_feadb, feat_feadb,
                  pos_w1, pos_b1, pos_g1, pos_be1, pos_w2, pos_b2,
                  attn_w1, attn_b1, attn_g1, attn_be1, attn_w2, attn_b2):
    f32 = np.float32
    bf16 = mybir.dt.np(BF16)
    a = {k: np.ascontiguousarray(np.asarray(v), dtype=f32) for k, v in dict(
        pcd=pcd, feat=feat, pcd_feadb=pcd_feadb, feat_feadb=feat_feadb,
        pos_w1=pos_w1, pos_b1=pos_b1, pos_g1=pos_g1, pos_be1=pos_be1,
        pos_w2=pos_w2, pos_b2=pos_b2,
        attn_w1=attn_w1, attn_b1=attn_b1, attn_g1=attn_g1, attn_be1=attn_be1,
        attn_w2=attn_w2, attn_b2=attn_b2).items()}

    fus_pcd = np.concatenate([a['pcd'], a['pcd_feadb']], axis=2)    # [B,3,F]
    fus_feat = np.concatenate([a['feat'], a['feat_feadb']], axis=2)  # [B,C,F]

    # BatchNorm (eval, running stats 0/1) folded into the conv weights.
    sp = (a['pos_g1'].astype(np.float64) / np.sqrt(1.0 + BN_EPS))
    w1p = a['pos_w1'].astype(np.float64) * sp[:, None]
    b1p = a['pos_b1'].astype(np.float64) * sp + a['pos_be1']
    sa = (a['attn_g1'].astype(np.float64) / np.sqrt(1.0 + BN_EPS))
    w1a = a['attn_w1'].astype(np.float64) * sa[:, None]
    # pos_b2 is applied on-chip as the pe-copy bias (pe == reference
    # pos_embedding); attn_b2 is dropped (softmax-invariant).
    b1a = a['attn_b1'].astype(np.float64) * sa + a['attn_be1']

    pos_w1t = np.zeros((4, PH), bf16)
    pos_w1t[:3] = w1p.T.astype(bf16)
    pos_b1v = b1p.astype(f32).reshape(PH, 1)
    pos_w2t = np.ascontiguousarray(a['pos_w2'].T).astype(bf16)
    pos_b2c = np.ascontiguousarray(a['pos_b2'].reshape(2, P).T)
    attn_w1t = np.ascontiguousarray(w1a.T).astype(bf16)
    attn_b1c = np.ascontiguousarray(b1a.astype(f32).reshape(AH // P, P).T)
    attn_w2t = np.ascontiguousarray(a['attn_w2'].T).astype(bf16)

    def split3(x):
        h = x.astype(bf16).astype(f32)
        r = x - h
        m = r.astype(bf16).astype(f32)
        lo = (r - m).astype(bf16).astype(f32)
        return h, m, lo

    per_batch = []
    for b in range(B):
        # 21-row bf16 split of S[f] = sum_d 2k_d f_d - |f|^2 (see v1).
        fus = fus_pcd[b].astype(f32)
        fh, fm, fl = split3(fus)                    # [3, F] each
        n64 = -np.sum(fus_pcd[b].astype(np.float64) ** 2, axis=0)
        nh, nm, nl = split3(n64.astype(f32) * 0 + n64)  # split fp64 value
        knn_rhs = np.zeros((21, F), bf16)
        r = 0
        rhs_rows = []
        for dd in range(3):
            rhs_rows += [(fh[dd], 'h', dd), (fm[dd], 'm', dd),
                         (fl[dd], 'l', dd), (fh[dd], 'H2', dd),
                         (fm[dd], 'M2', dd), (fh[dd], 'H3', dd)]
        for row, _, _ in rhs_rows:
            knn_rhs[r] = row.astype(bf16)
            r += 1
        knn_rhs[18] = nh.astype(bf16)
        knn_rhs[19] = nm.astype(bf16)
        knn_rhs[20] = nl.astype(bf16)
        db = np.zeros((F, ROW), bf16)
        db[:, :C] = fus_feat[b].T.astype(bf16)
        db[:, C:C + 3] = fus_pcd[b].T.astype(bf16)
        per_batch.append((knn_rhs, np.ascontiguousarray(db)))

    in_maps = []
    for core in range(NCORES):
        b, s = divmod(core, NCORES // B)
        sh = slice(s * SHARD, (s + 1) * SHARD)
        k2 = 2.0 * a['pcd'][b][:, sh].astype(f32)
        kh, km, kl = split3(k2)
        keys2t = np.zeros((21, SHARD), bf16)
        r = 0
        for dd in range(3):
            for krow in (kh[dd], kh[dd], kh[dd], km[dd], km[dd], kl[dd]):
                keys2t[r] = krow.astype(bf16)
                r += 1
        keys2t[18] = 1.0
        keys2t[19] = 1.0
        keys2t[20] = 1.0
        pcd_sh = np.zeros((4, SHARD), bf16)
        pcd_sh[:3] = a['pcd'][b][:, sh].astype(bf16)
        in_maps.append(dict(
            keys2t=keys2t,
            knn_rhs=per_batch[b][0],
            db_rows=per_batch[b][1],
            feat_sh=np.ascontiguousarray(a['feat'][b][:, sh]).astype(bf16),
            pcd_sh=pcd_sh,
            pos_w1t=pos_w1t, pos_b1=pos_b1v, pos_w2t=pos_w2t, pos_b2c=pos_b2c,
            attn_w1t=attn_w1t, attn_b1c=attn_b1c,
            attn_w2t=attn_w2t,
        ))
    return in_maps


def kernel(**inputs):
    global LAST_RESULT
    nc = _get_nc()
    in_maps = _prep_in_maps(**inputs)
    res = bass_utils.run_bass_kernel_spmd(
        nc, in_maps, core_ids=list(range(NCORES)), trace=TRACE)
    LAST_RESULT = res
    out = np.empty((B, C, N), np.float32)
    for core in range(NCORES):
        b, s = divmod(core, NCORES // B)
        out[b][:, s * SHARD:(s + 1) * SHARD] = res.results[core]["out"]
    return out
